# revision 2
# baseline (speedup 1.0000x reference)
"""Trainium2 Bass kernel v2 for nn_Attention_49082886259369.

Computes, per batch b (one batch per NeuronCore, 8 cores data-parallel):
    fac  = tanh(k @ W + q @ U)            [S, D]
    s    = v^T @ fac                      [D, D]
    attn = softmax(s, axis=batch)         <- couples cores: AllReduce max + sum
    out  = v @ attn                       [S, D]

Precision strategy: the PE reads f32r (fp32 truncated to fp22) at FULL
bf16 rate when the moving free dim >= 256, so every GEMM runs single-pass:
  - mm1 (k@W, q@U) and mm2 (v^T@fac) in f32r: per-product error ~2^-14,
    measured end-to-end output error ~2.3e-3 (gate 2e-2).
  - mm3 (v@attn) in bf16 (attn in [0,1], error does not amplify).
  - AllReduce max AND sum both in bf16 (halves collective bytes).
This cuts physical PE passes 10 -> 4 vs the hi/lo-split baseline.

Structure (v3): phase 1 streams kT/qT and computes fac = tanh(k@W+q@U)
for ALL mi, storing fac (fp16) and v (fp16) resident in SBUF. Phase 2
then computes s per 256-col e-chunk (full 16-mi PSUM accumulation chains,
one DVE copy per (chunk,di)) so the first AllReduce-max launches ~40us
earlier and the CC chain (the serial resource) overlaps phase-2 compute.
AR-max runs in fp8e4m3 (max only needs ~8-unit accuracy for exp range
control), e staging and AR-sum in fp16.

Layouts pre-tiled on host so every DMA is [128 partitions x contiguous]:
  kT/qT: [MT, 128, DT, 128] f32 with [mi,p,di,sj] = x[mi*128+sj, di*128+p]
  W/U:   [128, DT, D]       f32 with [p,di,e]     = W[di*128+p, e]
  v:     [MT, 128, D]       f32 with [mi,p,d]     = v[mi*128+p, d]
  vT:    [MT, 128, DT, 128] bf16, same index map as kT
"""

import os

import numpy as np
import ml_dtypes

B, S, D = 8, 2048, 1024
NCORES = 8
P = 128
NE = 512   # matmul free-dim tile (one PSUM bank of fp32)
CW = 256   # softmax chunk width
G = 4      # mi-group size for s-accumulation in PSUM

_CACHE: dict = {}


# --------------------------------------------------------------------------
# device kernel builder
# --------------------------------------------------------------------------

def _build_nc(s_dim: int, d_dim: int, n_reps: int = 1, variant: str = "full"):
    import concourse.mybir as mybir
    import concourse.tile as tile
    from concourse import bacc

    F32 = mybir.dt.float32
    F32R = mybir.dt.float32r
    BF16 = mybir.dt.bfloat16
    FP16 = mybir.dt.float16
    FP8 = mybir.dt.float8e4
    ACT = mybir.ActivationFunctionType

    MT = s_dim // P          # 16 row tiles of S
    DT = d_dim // P          # 8 row tiles of D
    NH = d_dim // NE         # 2 e-halves (psum bank granularity for mm1)
    NCH = d_dim // CW        # 4 softmax chunks

    nc = bacc.Bacc("TRN2", target_bir_lowering=False, num_devices=NCORES)

    d_kT = nc.dram_tensor("kT", [MT, P, DT, P], F32R, kind="ExternalInput")
    d_qT = nc.dram_tensor("qT", [MT, P, DT, P], F32R, kind="ExternalInput")
    d_W = nc.dram_tensor("W", [P, DT, d_dim], F32R, kind="ExternalInput")
    d_U = nc.dram_tensor("U", [P, DT, d_dim], F32R, kind="ExternalInput")
    d_v16 = nc.dram_tensor("v16", [MT, P, d_dim], FP16, kind="ExternalInput")
    d_vT = nc.dram_tensor("vT", [MT, P, DT, P], BF16, kind="ExternalInput")
    d_out = nc.dram_tensor("out", [s_dim, d_dim], F32, kind="ExternalOutput")

    with tile.TileContext(nc) as tc:
        with (
            tc.tile_pool(name="wu", bufs=1) as wu_pool,
            tc.tile_pool(name="kq", bufs=2) as kq_pool,
            tc.tile_pool(name="vres", bufs=1) as v_pool,
            tc.tile_pool(name="fac", bufs=1) as fac_pool,
            tc.tile_pool(name="spers", bufs=1) as s_pool,
            tc.tile_pool(name="vt", bufs=2) as vt_pool,
            tc.tile_pool(name="stg", bufs=1) as stg_pool,
            tc.tile_pool(name="stat", bufs=1) as stat_pool,
            tc.tile_pool(name="att", bufs=1) as att_pool,
            tc.tile_pool(name="ost", bufs=2) as out_pool,
            tc.tile_pool(name="fps", bufs=2, space="PSUM") as fac_psum,
            tc.tile_pool(name="sps", bufs=4, space="PSUM") as s_psum,
            tc.tile_pool(name="ops", bufs=2, space="PSUM") as out_psum,
            tc.tile_pool(name="dram", bufs=1, space="DRAM") as dram_pool,
        ):
          for _rep in range(n_reps):
            # resident tensors
            fac_res = fac_pool.tile([P, MT, d_dim], FP16, tag="fac",
                                    name="fac")
            v_res = v_pool.tile([P, MT, d_dim], FP16, tag="vres",
                                name="vres")
            s_t = [
                s_pool.tile([P, DT, CW], F32, tag=f"s{c}", name=f"s{c}")
                for c in range(NCH)
            ]

            # prefetch mi=0 operands ahead of the bulky W/U loads
            kt0 = kq_pool.tile([P, DT, P], F32R, tag="kt", name="ktp")
            qt0 = kq_pool.tile([P, DT, P], F32R, tag="qt", name="qtp")
            nc.sync.dma_start(out=kt0, in_=d_kT[0])
            nc.sync.dma_start(out=qt0, in_=d_qT[0])

            w_all = wu_pool.tile([P, DT, d_dim], F32R, tag="w", name="w")
            u_all = wu_pool.tile([P, DT, d_dim], F32R, tag="u", name="u")
            for di in range(DT):
                nc.sync.dma_start(out=w_all[:, di, :], in_=d_W[:, di, :])
                nc.sync.dma_start(out=u_all[:, di, :], in_=d_U[:, di, :])

            # ---- phase 1: mm1 + tanh -> fac_res (fp16), v_res loads ----
            for mi in range(MT):
                if mi == 0:
                    kt, qt = kt0, qt0
                else:
                    kt = kq_pool.tile([P, DT, P], F32R, tag="kt")
                    qt = kq_pool.tile([P, DT, P], F32R, tag="qt")
                    nc.sync.dma_start(out=kt, in_=d_kT[mi])
                    nc.sync.dma_start(out=qt, in_=d_qT[mi])
                nc.sync.dma_start(out=v_res[:, mi, :], in_=d_v16[mi])
                for ni in range(NH):
                    esl = slice(ni * NE, (ni + 1) * NE)
                    fps = fac_psum.tile([P, NE], F32)
                    for di in range(DT):
                        nc.tensor.matmul(fps, kt[:, di, :],
                                         w_all[:, di, esl],
                                         start=(di == 0), stop=False)
                    for di in range(DT):
                        nc.tensor.matmul(fps, qt[:, di, :],
                                         u_all[:, di, esl],
                                         start=False, stop=(di == DT - 1))
                    nc.scalar.activation(fac_res[:, mi, esl], fps, ACT.Tanh)

            # ---- phase 2: mm2 per 256-col chunk + AR-max (fp8) ASAP ----
            # CC issue order interleaves maxes and sums so the serial CC
            # resource is never idle while a dependency is pending.
            cmax = [None] * NCH
            csum = [None] * NCH
            e_t = [None] * NCH

            def issue_max(c):
                sb = stg_pool.tile([P, DT, CW], FP8, tag="sb")
                with nc.allow_low_precision(reason="fp8 max proxy"):
                    nc.vector.tensor_copy(sb, s_t[c])
                cin = dram_pool.tile([P, DT, CW], FP8, tag=f"cmi{c}",
                                     name=f"cmi{c}")
                cmo = dram_pool.tile([P, DT, CW], FP8, tag=f"cmo{c}",
                                     name=f"cmo{c}", addr_space="Shared")
                nc.sync.dma_start(out=cin, in_=sb)
                if variant != "nocc":
                    nc.gpsimd.collective_compute(
                        "AllReduce", mybir.AluOpType.max,
                        replica_groups=[list(range(NCORES))],
                        ins=[cin.opt()], outs=[cmo.opt()],
                    )
                else:
                    nc.gpsimd.dma_start(out=cmo[:], in_=cin[:])
                cmax[c] = cmo

            def issue_sum(c):
                cm = tc.tile_wait_until(0.135 + 0.022 * c)
                cm.__enter__()
                m8 = stat_pool.tile([P, DT, CW], FP8, tag="m8")
                nc.sync.dma_start(out=m8, in_=cmax[c])
                m16 = stat_pool.tile([P, DT, CW], FP16, tag="m16")
                with nc.allow_low_precision(reason="max proxy fp16"):
                    nc.vector.tensor_copy(m16, m8)
                nc.vector.tensor_sub(s_t[c], s_t[c], m16)
                nc.scalar.activation(s_t[c], s_t[c], ACT.Exp)
                e16 = stg_pool.tile([P, DT, CW], FP16, tag="e16")
                with nc.allow_low_precision(reason="e staging fp16"):
                    nc.vector.tensor_copy(e16, s_t[c])
                ein = dram_pool.tile([P, DT, CW], FP16, tag=f"cei{c}",
                                     name=f"cei{c}")
                eout = dram_pool.tile([P, DT, CW], FP16, tag=f"ceo{c}",
                                      name=f"ceo{c}", addr_space="Shared")
                nc.sync.dma_start(out=ein, in_=e16)
                if variant != "nocc":
                    nc.gpsimd.collective_compute(
                        "AllReduce", mybir.AluOpType.add,
                        replica_groups=[list(range(NCORES))],
                        ins=[ein.opt()], outs=[eout.opt()],
                    )
                else:
                    nc.gpsimd.dma_start(out=eout[:], in_=ein[:])
                csum[c] = eout
                cm.__exit__(None, None, None)

            def mm2_chunk(c):
                csl = slice(c * CW, (c + 1) * CW)
                for di in range(DT):
                    sps = s_psum.tile([P, CW], F32)
                    for mi in range(MT):
                        nc.tensor.matmul(
                            sps, v_res[:, mi, di * P:(di + 1) * P],
                            fac_res[:, mi, csl],
                            start=(mi == 0), stop=(mi == MT - 1),
                        )
                    nc.vector.tensor_copy(s_t[c][:, di, :], sps)

            # schedule: mm2 c; max c as soon as s_t[c] done; sums interleaved
            for c in range(NCH):
                mm2_chunk(c)
                issue_max(c)
                if c >= 1:
                    issue_sum(c - 1)
            issue_sum(NCH - 1)

            # ---- normalize + mm3 per chunk ----
            for c in range(NCH):
                cm3 = tc.tile_wait_until(0.168 + 0.022 * c)
                cm3.__enter__()
                gsl = slice(c * CW, (c + 1) * CW)
                den16 = stat_pool.tile([P, DT, CW], FP16, tag="den16")
                nc.sync.dma_start(out=den16, in_=csum[c])
                with nc.allow_low_precision(reason="softmax denom fp16"):
                    nc.vector.reciprocal(den16, den16)
                    att = att_pool.tile([P, DT, CW], BF16, tag=f"att{c % 2}")
                    nc.vector.tensor_mul(att, s_t[c], den16)
                for mi in range(MT):
                    vt = vt_pool.tile([P, DT, P], BF16, tag="vt")
                    nc.sync.dma_start(out=vt, in_=d_vT[mi])
                    ops = out_psum.tile([P, CW], F32)
                    for di in range(DT):
                        nc.tensor.matmul(
                            ops, vt[:, di, :], att[:, di, :],
                            start=(di == 0), stop=(di == DT - 1),
                        )
                    ost = out_pool.tile([P, CW], F32, tag="ost")
                    nc.vector.tensor_copy(ost, ops)
                    nc.sync.dma_start(
                        out=d_out[mi * P:(mi + 1) * P, gsl], in_=ost
                    )
                cm3.__exit__(None, None, None)
            tc.tile_update_base_wait()

    nc.compile()
    return nc


def _get_nc(s_dim=S, d_dim=D, n_reps=1, variant="full"):
    key = ("nc", s_dim, d_dim, n_reps, variant)
    if key not in _CACHE:
        _CACHE[key] = _build_nc(s_dim, d_dim, n_reps, variant)
    return _CACHE[key]


# --------------------------------------------------------------------------
# host-side packing
# --------------------------------------------------------------------------

def _tileT(x: np.ndarray, s_dim: int, d_dim: int) -> np.ndarray:
    """[S, D] -> [MT, 128, DT, 128] with [mi,p,di,sj] = x[mi*128+sj, di*128+p]."""
    mt, dt = s_dim // P, d_dim // P
    return np.ascontiguousarray(
        x.reshape(mt, P, dt, P).transpose(0, 3, 2, 1)
    )


def prepare_in_maps(q, k, v, W, U, s_dim=S, d_dim=D):
    q = np.asarray(q, dtype=np.float32)
    k = np.asarray(k, dtype=np.float32)
    v = np.asarray(v, dtype=np.float32)
    W = np.asarray(W, dtype=np.float32)
    U = np.asarray(U, dtype=np.float32)

    dt = d_dim // P
    mt = s_dim // P
    W_t = np.ascontiguousarray(W.reshape(dt, P, d_dim).transpose(1, 0, 2))
    U_t = np.ascontiguousarray(U.reshape(dt, P, d_dim).transpose(1, 0, 2))

    in_maps = []
    for b in range(NCORES):
        in_maps.append({
            "kT": _tileT(k[b], s_dim, d_dim),
            "qT": _tileT(q[b], s_dim, d_dim),
            "W": W_t, "U": U_t,
            "v16": v[b].reshape(mt, P, d_dim).astype(np.float16),
            "vT": _tileT(v[b], s_dim, d_dim).astype(ml_dtypes.bfloat16),
        })
    return in_maps


def run_spmd(in_maps, s_dim=S, d_dim=D):
    """One-shot path through the stock bass_utils helper (debug use)."""
    from concourse import bass_utils
    nc = _get_nc(s_dim, d_dim)
    res = bass_utils.run_bass_kernel_spmd(
        nc, in_maps=in_maps, core_ids=list(range(NCORES))
    )
    return res


def _get_runner(s_dim=S, d_dim=D, n_reps=1, variant="full"):
    """Cached sharded-jit runner over the same bass2jax/_bass_exec_p path
    that bass_utils.run_bass_kernel_spmd uses under axon, but built once per
    process (no donation) so repeat calls skip re-trace/re-compile."""
    key = ("runner", s_dim, d_dim, n_reps, variant)
    if key in _CACHE:
        return _CACHE[key]

    import jax
    from jax.sharding import Mesh, PartitionSpec
    from jax.experimental.shard_map import shard_map
    import concourse.mybir as mybir
    from concourse import bass2jax

    nc = _get_nc(s_dim, d_dim, n_reps, variant)
    bass2jax.install_neuronx_cc_hook()

    partition_name = (
        nc.partition_id_tensor.name if nc.partition_id_tensor else None
    )
    in_names, out_names, out_avals, zero_outs = [], [], [], []
    for alloc in nc.m.functions[0].allocations:
        if not isinstance(alloc, mybir.MemoryLocationSet):
            continue
        name = alloc.memorylocations[0].name
        if alloc.kind == "ExternalInput":
            if name != partition_name:
                in_names.append(name)
        elif alloc.kind == "ExternalOutput":
            shape = tuple(alloc.tensor_shape)
            dtype = mybir.dt.np(alloc.dtype)
            out_names.append(name)
            out_avals.append(jax.core.ShapedArray(shape, dtype))
            zero_outs.append(np.zeros(shape, dtype))
    n_params = len(in_names)
    all_in_names = list(in_names) + list(out_names)
    if partition_name is not None:
        all_in_names.append(partition_name)

    def _body(*args):
        operands = list(args)
        if partition_name is not None:
            operands.append(bass2jax.partition_id_tensor())
        outs = bass2jax._bass_exec_p.bind(
            *operands,
            out_avals=tuple(out_avals),
            in_names=tuple(all_in_names),
            out_names=tuple(out_names),
            lowering_input_output_aliases=(),
            sim_require_finite=True,
            sim_require_nnan=True,
            nc=nc,
        )
        return tuple(outs)

    devices = jax.devices()[:NCORES]
    mesh = Mesh(np.asarray(devices), ("core",))
    in_specs = (PartitionSpec("core"),) * (n_params + len(out_names))
    out_specs = (PartitionSpec("core"),) * len(out_names)
    sharded = jax.jit(
        shard_map(
            _body, mesh=mesh, in_specs=in_specs, out_specs=out_specs,
            check_rep=False,
        ),
        keep_unused=True,
    )
    runner = {
        "fn": sharded,
        "in_names": in_names,
        "out_names": out_names,
        "out_avals": out_avals,
        "zero_concat": [
            np.zeros((NCORES * z.shape[0], *z.shape[1:]), z.dtype)
            for z in zero_outs
        ],
        "mesh": mesh,
    }
    _CACHE[key] = runner
    return runner


def _concat_inputs(runner, in_maps):
    return [
        np.concatenate([np.asarray(m[name]) for m in in_maps], axis=0)
        for name in runner["in_names"]
    ]


def run_fast(in_maps, s_dim=S, d_dim=D):
    """Execute via the cached runner; returns list of per-core out dicts."""
    runner = _get_runner(s_dim, d_dim)
    concat_in = _concat_inputs(runner, in_maps)
    out_arrs = runner["fn"](*concat_in, *runner["zero_concat"])
    results = []
    for c in range(NCORES):
        results.append({
            name: np.asarray(out_arrs[i]).reshape(
                NCORES, *runner["out_avals"][i].shape
            )[c]
            for i, name in enumerate(runner["out_names"])
        })
    return results


def kernel(q, k, v, W, U):
    in_maps = prepare_in_maps(q, k, v, W, U)
    if os.environ.get("BASS_USE_SPMD_HELPER"):
        res = run_spmd(in_maps)
        results = res.results
    else:
        results = run_fast(in_maps)
    out = np.stack([results[b]["out"] for b in range(NCORES)], axis=0)
    return out.astype(np.float32)


# revision 3
# speedup vs baseline: 1.2422x; 1.2422x over previous
"""Trainium2 Bass kernel v2 for nn_Attention_49082886259369.

Computes, per batch b (one batch per NeuronCore, 8 cores data-parallel):
    fac  = tanh(k @ W + q @ U)            [S, D]
    s    = v^T @ fac                      [D, D]
    attn = softmax(s, axis=batch)         <- couples cores: AllReduce max + sum
    out  = v @ attn                       [S, D]

Precision strategy: the PE reads f32r (fp32 truncated to fp22) at FULL
bf16 rate when the moving free dim >= 256, so every GEMM runs single-pass:
  - mm1 (k@W, q@U) and mm2 (v^T@fac) in f32r: per-product error ~2^-14,
    measured end-to-end output error ~2.3e-3 (gate 2e-2).
  - mm3 (v@attn) in bf16 (attn in [0,1], error does not amplify).
  - AllReduce max AND sum both in bf16 (halves collective bytes).
This cuts physical PE passes 10 -> 4 vs the hi/lo-split baseline.

Structure (v3): phase 1 streams kT/qT and computes fac = tanh(k@W+q@U)
for ALL mi, storing fac (fp16) and v (fp16) resident in SBUF. Phase 2
then computes s per 256-col e-chunk (full 16-mi PSUM accumulation chains,
one DVE copy per (chunk,di)) so the first AllReduce-max launches ~40us
earlier and the CC chain (the serial resource) overlaps phase-2 compute.
AR-max runs in fp8e4m3 (max only needs ~8-unit accuracy for exp range
control), e staging and AR-sum in fp16.

Layouts pre-tiled on host so every DMA is [128 partitions x contiguous]:
  kT/qT: [MT, 128, DT, 128] f32 with [mi,p,di,sj] = x[mi*128+sj, di*128+p]
  W/U:   [128, DT, D]       f32 with [p,di,e]     = W[di*128+p, e]
  v:     [MT, 128, D]       f32 with [mi,p,d]     = v[mi*128+p, d]
  vT:    [MT, 128, DT, 128] bf16, same index map as kT
"""

import os

import numpy as np
import ml_dtypes

B, S, D = 8, 2048, 1024
NCORES = 8
P = 128
NE = 512   # matmul free-dim tile (one PSUM bank of fp32)
CW = 256   # softmax chunk width

_CACHE: dict = {}


# --------------------------------------------------------------------------
# device kernel builder
# --------------------------------------------------------------------------

def _build_nc(s_dim: int, d_dim: int, n_reps: int = 1, variant: str = "full"):
    import concourse.mybir as mybir
    import concourse.tile as tile
    from concourse import bacc

    F32 = mybir.dt.float32
    F32R = mybir.dt.float32r
    BF16 = mybir.dt.bfloat16
    FP16 = mybir.dt.float16
    FP8 = mybir.dt.float8e4
    ACT = mybir.ActivationFunctionType

    MT = s_dim // P          # 16 row tiles of S
    DT = d_dim // P          # 8 row tiles of D
    NH = d_dim // NE         # 2 e-halves (psum bank granularity for mm1)
    NCH = d_dim // CW        # 4 softmax chunks

    nc = bacc.Bacc("TRN2", target_bir_lowering=False, num_devices=NCORES)

    d_kT = nc.dram_tensor("kT", [MT, P, DT, P], F32R, kind="ExternalInput")
    d_qT = nc.dram_tensor("qT", [MT, P, DT, P], F32R, kind="ExternalInput")
    d_W = nc.dram_tensor("W", [P, DT, d_dim], F32R, kind="ExternalInput")
    d_U = nc.dram_tensor("U", [P, DT, d_dim], F32R, kind="ExternalInput")
    d_v16 = nc.dram_tensor("v16", [MT, P, d_dim], FP16, kind="ExternalInput")
    d_vT = nc.dram_tensor("vT", [MT, P, DT, P], BF16, kind="ExternalInput")
    d_out = nc.dram_tensor("out", [s_dim, d_dim], F32, kind="ExternalOutput")

    with tile.TileContext(nc) as tc:
        with (
            tc.tile_pool(name="wu", bufs=1) as wu_pool,
            tc.tile_pool(name="kq", bufs=2) as kq_pool,
            tc.tile_pool(name="vres", bufs=1) as v_pool,
            tc.tile_pool(name="fac", bufs=1) as fac_pool,
            tc.tile_pool(name="spers", bufs=1) as s_pool,
            tc.tile_pool(name="vt", bufs=2) as vt_pool,
            tc.tile_pool(name="stg", bufs=1) as stg_pool,
            tc.tile_pool(name="stat", bufs=1) as stat_pool,
            tc.tile_pool(name="att", bufs=1) as att_pool,
            tc.tile_pool(name="ost", bufs=2) as out_pool,
            tc.tile_pool(name="fps", bufs=2, space="PSUM") as fac_psum,
            tc.tile_pool(name="sps", bufs=4, space="PSUM") as s_psum,
            tc.tile_pool(name="ops", bufs=2, space="PSUM") as out_psum,
            tc.tile_pool(name="dram", bufs=1, space="DRAM") as dram_pool,
        ):
          for _rep in range(n_reps):
            # resident tensors
            fac_res = fac_pool.tile([P, MT, d_dim], FP16, tag="fac",
                                    name="fac")
            v_res = v_pool.tile([P, MT, d_dim], FP16, tag="vres",
                                name="vres")
            s_t = [
                s_pool.tile([P, DT, CW], F32, tag=f"s{c}", name=f"s{c}")
                for c in range(NCH)
            ]

            # prefetch mi=0 operands ahead of the bulky W/U loads
            kt0 = kq_pool.tile([P, DT, P], F32R, tag="kt", name="ktp")
            qt0 = kq_pool.tile([P, DT, P], F32R, tag="qt", name="qtp")
            nc.sync.dma_start(out=kt0, in_=d_kT[0])
            nc.sync.dma_start(out=qt0, in_=d_qT[0])

            w_all = wu_pool.tile([P, DT, d_dim], F32R, tag="w", name="w")
            u_all = wu_pool.tile([P, DT, d_dim], F32R, tag="u", name="u")
            for di in range(DT):
                nc.sync.dma_start(out=w_all[:, di, :], in_=d_W[:, di, :])
                nc.sync.dma_start(out=u_all[:, di, :], in_=d_U[:, di, :])

            # ---- phase 1: mm1 + tanh -> fac_res (fp16), v_res loads ----
            for mi in range(MT):
                if mi == 0:
                    kt, qt = kt0, qt0
                else:
                    kt = kq_pool.tile([P, DT, P], F32R, tag="kt")
                    qt = kq_pool.tile([P, DT, P], F32R, tag="qt")
                    nc.sync.dma_start(out=kt, in_=d_kT[mi])
                    nc.sync.dma_start(out=qt, in_=d_qT[mi])
                nc.sync.dma_start(out=v_res[:, mi, :], in_=d_v16[mi])
                for ni in range(NH):
                    esl = slice(ni * NE, (ni + 1) * NE)
                    fps = fac_psum.tile([P, NE], F32)
                    for di in range(DT):
                        nc.tensor.matmul(fps, kt[:, di, :],
                                         w_all[:, di, esl],
                                         start=(di == 0), stop=False)
                    for di in range(DT):
                        nc.tensor.matmul(fps, qt[:, di, :],
                                         u_all[:, di, esl],
                                         start=False, stop=(di == DT - 1))
                    nc.scalar.activation(fac_res[:, mi, esl], fps, ACT.Tanh)

            # ---- phase 2: mm2 per 256-col chunk + AR-max (fp8) ASAP ----
            # CC issue order interleaves maxes and sums so the serial CC
            # resource is never idle while a dependency is pending.
            cmax = [None] * NCH
            csum = [None] * NCH
            e_t = [None] * NCH

            def issue_max(c):
                sb = stg_pool.tile([P, DT, CW], FP8, tag="sb")
                with nc.allow_low_precision(reason="fp8 max proxy"):
                    nc.vector.tensor_copy(sb, s_t[c])
                cin = dram_pool.tile([P, DT, CW], FP8, tag=f"cmi{c}",
                                     name=f"cmi{c}")
                cmo = dram_pool.tile([P, DT, CW], FP8, tag=f"cmo{c}",
                                     name=f"cmo{c}", addr_space="Shared")
                nc.sync.dma_start(out=cin, in_=sb)
                if variant != "nocc":
                    nc.gpsimd.collective_compute(
                        "AllReduce", mybir.AluOpType.max,
                        replica_groups=[list(range(NCORES))],
                        ins=[cin.opt()], outs=[cmo.opt()],
                    )
                else:
                    nc.gpsimd.dma_start(out=cmo[:], in_=cin[:])
                cmax[c] = cmo

            def issue_sum(c):
                cm = tc.tile_wait_until(0.135 + 0.022 * c)
                cm.__enter__()
                m8 = stat_pool.tile([P, DT, CW], FP8, tag="m8")
                nc.sync.dma_start(out=m8, in_=cmax[c])
                m16 = stat_pool.tile([P, DT, CW], FP16, tag="m16")
                with nc.allow_low_precision(reason="max proxy fp16"):
                    nc.vector.tensor_copy(m16, m8)
                nc.vector.tensor_sub(s_t[c], s_t[c], m16)
                nc.scalar.activation(s_t[c], s_t[c], ACT.Exp)
                e16 = stg_pool.tile([P, DT, CW], FP16, tag="e16")
                with nc.allow_low_precision(reason="e staging fp16"):
                    nc.vector.tensor_copy(e16, s_t[c])
                ein = dram_pool.tile([P, DT, CW], FP16, tag=f"cei{c}",
                                     name=f"cei{c}")
                eout = dram_pool.tile([P, DT, CW], FP16, tag=f"ceo{c}",
                                      name=f"ceo{c}", addr_space="Shared")
                nc.sync.dma_start(out=ein, in_=e16)
                if variant != "nocc":
                    nc.gpsimd.collective_compute(
                        "AllReduce", mybir.AluOpType.add,
                        replica_groups=[list(range(NCORES))],
                        ins=[ein.opt()], outs=[eout.opt()],
                    )
                else:
                    nc.gpsimd.dma_start(out=eout[:], in_=ein[:])
                csum[c] = eout
                cm.__exit__(None, None, None)

            def mm2_chunk(c):
                csl = slice(c * CW, (c + 1) * CW)
                for di in range(DT):
                    sps = s_psum.tile([P, CW], F32)
                    for mi in range(MT):
                        nc.tensor.matmul(
                            sps, v_res[:, mi, di * P:(di + 1) * P],
                            fac_res[:, mi, csl],
                            start=(mi == 0), stop=(mi == MT - 1),
                        )
                    nc.vector.tensor_copy(s_t[c][:, di, :], sps)

            # schedule: mm2 c; max c as soon as s_t[c] done; sums interleaved
            for c in range(NCH):
                mm2_chunk(c)
                issue_max(c)
                if c >= 1:
                    issue_sum(c - 1)
            issue_sum(NCH - 1)

            # ---- normalize + mm3 per chunk ----
            for c in range(NCH):
                cm3 = tc.tile_wait_until(0.168 + 0.022 * c)
                cm3.__enter__()
                gsl = slice(c * CW, (c + 1) * CW)
                den16 = stat_pool.tile([P, DT, CW], FP16, tag="den16")
                nc.sync.dma_start(out=den16, in_=csum[c])
                with nc.allow_low_precision(reason="softmax denom fp16"):
                    nc.vector.reciprocal(den16, den16)
                    att = att_pool.tile([P, DT, CW], BF16, tag=f"att{c % 2}")
                    nc.vector.tensor_mul(att, s_t[c], den16)
                for mi in range(MT):
                    vt = vt_pool.tile([P, DT, P], BF16, tag="vt")
                    nc.sync.dma_start(out=vt, in_=d_vT[mi])
                    ops = out_psum.tile([P, CW], F32)
                    for di in range(DT):
                        nc.tensor.matmul(
                            ops, vt[:, di, :], att[:, di, :],
                            start=(di == 0), stop=(di == DT - 1),
                        )
                    ost = out_pool.tile([P, CW], F32, tag="ost")
                    nc.vector.tensor_copy(ost, ops)
                    nc.sync.dma_start(
                        out=d_out[mi * P:(mi + 1) * P, gsl], in_=ost
                    )
                cm3.__exit__(None, None, None)
            tc.tile_update_base_wait()

    nc.compile()
    return nc


def _get_nc(s_dim=S, d_dim=D, n_reps=1, variant="full"):
    key = ("nc", s_dim, d_dim, n_reps, variant)
    if key not in _CACHE:
        _CACHE[key] = _build_nc(s_dim, d_dim, n_reps, variant)
    return _CACHE[key]


# --------------------------------------------------------------------------
# host-side packing
# --------------------------------------------------------------------------

def _tileT(x: np.ndarray, s_dim: int, d_dim: int) -> np.ndarray:
    """[S, D] -> [MT, 128, DT, 128] with [mi,p,di,sj] = x[mi*128+sj, di*128+p]."""
    mt, dt = s_dim // P, d_dim // P
    return np.ascontiguousarray(
        x.reshape(mt, P, dt, P).transpose(0, 3, 2, 1)
    )


def prepare_in_maps(q, k, v, W, U, s_dim=S, d_dim=D):
    q = np.asarray(q, dtype=np.float32)
    k = np.asarray(k, dtype=np.float32)
    v = np.asarray(v, dtype=np.float32)
    W = np.asarray(W, dtype=np.float32)
    U = np.asarray(U, dtype=np.float32)

    dt = d_dim // P
    mt = s_dim // P
    W_t = np.ascontiguousarray(W.reshape(dt, P, d_dim).transpose(1, 0, 2))
    U_t = np.ascontiguousarray(U.reshape(dt, P, d_dim).transpose(1, 0, 2))

    in_maps = []
    for b in range(NCORES):
        in_maps.append({
            "kT": _tileT(k[b], s_dim, d_dim),
            "qT": _tileT(q[b], s_dim, d_dim),
            "W": W_t, "U": U_t,
            "v16": v[b].reshape(mt, P, d_dim).astype(np.float16),
            "vT": _tileT(v[b], s_dim, d_dim).astype(ml_dtypes.bfloat16),
        })
    return in_maps


def run_spmd(in_maps, s_dim=S, d_dim=D):
    """One-shot path through the stock bass_utils helper (debug use)."""
    from concourse import bass_utils
    nc = _get_nc(s_dim, d_dim)
    res = bass_utils.run_bass_kernel_spmd(
        nc, in_maps=in_maps, core_ids=list(range(NCORES))
    )
    return res


def _get_runner(s_dim=S, d_dim=D, n_reps=1, variant="full"):
    """Cached sharded-jit runner over the same bass2jax/_bass_exec_p path
    that bass_utils.run_bass_kernel_spmd uses under axon, but built once per
    process (no donation) so repeat calls skip re-trace/re-compile."""
    key = ("runner", s_dim, d_dim, n_reps, variant)
    if key in _CACHE:
        return _CACHE[key]

    import jax
    from jax.sharding import Mesh, PartitionSpec
    from jax.experimental.shard_map import shard_map
    import concourse.mybir as mybir
    from concourse import bass2jax

    nc = _get_nc(s_dim, d_dim, n_reps, variant)
    bass2jax.install_neuronx_cc_hook()

    partition_name = (
        nc.partition_id_tensor.name if nc.partition_id_tensor else None
    )
    in_names, out_names, out_avals, zero_outs = [], [], [], []
    for alloc in nc.m.functions[0].allocations:
        if not isinstance(alloc, mybir.MemoryLocationSet):
            continue
        name = alloc.memorylocations[0].name
        if alloc.kind == "ExternalInput":
            if name != partition_name:
                in_names.append(name)
        elif alloc.kind == "ExternalOutput":
            shape = tuple(alloc.tensor_shape)
            dtype = mybir.dt.np(alloc.dtype)
            out_names.append(name)
            out_avals.append(jax.core.ShapedArray(shape, dtype))
            zero_outs.append(np.zeros(shape, dtype))
    n_params = len(in_names)
    all_in_names = list(in_names) + list(out_names)
    if partition_name is not None:
        all_in_names.append(partition_name)

    def _body(*args):
        operands = list(args)
        if partition_name is not None:
            operands.append(bass2jax.partition_id_tensor())
        outs = bass2jax._bass_exec_p.bind(
            *operands,
            out_avals=tuple(out_avals),
            in_names=tuple(all_in_names),
            out_names=tuple(out_names),
            lowering_input_output_aliases=(),
            sim_require_finite=True,
            sim_require_nnan=True,
            nc=nc,
        )
        return tuple(outs)

    devices = jax.devices()[:NCORES]
    mesh = Mesh(np.asarray(devices), ("core",))
    in_specs = (PartitionSpec("core"),) * (n_params + len(out_names))
    out_specs = (PartitionSpec("core"),) * len(out_names)
    sharded = jax.jit(
        shard_map(
            _body, mesh=mesh, in_specs=in_specs, out_specs=out_specs,
            check_rep=False,
        ),
        keep_unused=True,
    )
    runner = {
        "fn": sharded,
        "in_names": in_names,
        "out_names": out_names,
        "out_avals": out_avals,
        "zero_concat": [
            np.zeros((NCORES * z.shape[0], *z.shape[1:]), z.dtype)
            for z in zero_outs
        ],
        "mesh": mesh,
    }
    _CACHE[key] = runner
    return runner


def _concat_inputs(runner, in_maps):
    return [
        np.concatenate([np.asarray(m[name]) for m in in_maps], axis=0)
        for name in runner["in_names"]
    ]


def run_fast(in_maps, s_dim=S, d_dim=D):
    """Execute via the cached runner; returns list of per-core out dicts."""
    runner = _get_runner(s_dim, d_dim)
    concat_in = _concat_inputs(runner, in_maps)
    out_arrs = runner["fn"](*concat_in, *runner["zero_concat"])
    results = []
    for c in range(NCORES):
        results.append({
            name: np.asarray(out_arrs[i]).reshape(
                NCORES, *runner["out_avals"][i].shape
            )[c]
            for i, name in enumerate(runner["out_names"])
        })
    return results


def kernel(q, k, v, W, U):
    in_maps = prepare_in_maps(q, k, v, W, U)
    if os.environ.get("BASS_USE_SPMD_HELPER"):
        res = run_spmd(in_maps)
        results = res.results
    else:
        results = run_fast(in_maps)
    out = np.stack([results[b]["out"] for b in range(NCORES)], axis=0)
    return out.astype(np.float32)
